# revision 1
# baseline (speedup 1.0000x reference)
"""CausalSelfAttention TRN2 kernel: LN + QKV + causal attention + out_proj.

Sharding: 8 cores = 4 batches x 2 head-groups (8 heads each). Each core
computes its batch's LayerNorm, QKV for its heads, causal softmax attention,
and a partial out-projection over its heads' channels; the host sums the two
partials per batch.

Per-core layouts (SBUF partition dim first):
  hT   [c, t]   LN(x) transposed via PE, bf16
  qT/kT [o, t]  o = head*64+d; head pair (2i,2i+1) shares a 128-partition tile
  v    [t, (h, 65)] bf16, col 64 = ones (PV emits softmax sums as row 64)
  scores sT [tk, tq] per 128x512 tile, K=64 head-pairs row-tiled concurrently;
  exp on ACT (scale=1/8 fused) over valid columns only (tq_loc >= r for the
  diagonal tile at offset r); causality via a single 128-wide multiplicative
  [i>j] mask on DVE; PV: lhsT=v_ext, rhs=p[:, r:] -> out2t [65, tq]
  normalization: sums -> DRAM roundtrip -> 64-partition broadcast -> DVE mul
  out_proj: lhsT = A.T [j, t] f32r, rhs = woT [j, o] f32r
"""
import math
import sys

sys.path.insert(0, "/opt/trn_rl_repo")
sys.path.insert(0, "/opt/trn_rl_repo/concourse")

import numpy as np
import ml_dtypes

import concourse.bass as bass
import concourse.bacc as bacc
import concourse.mybir as mybir
import concourse.tile as tile
from concourse.bass_utils import run_bass_kernel_spmd

T, C, NH, DH = 2048, 1024, 16, 64
HC = 8            # heads per core
NT = T // 128     # 16 t-tiles
KC = C // 128     # 8 contraction tiles
W = 512           # tq block width
NJ = T // W       # 4 q blocks
NP = HC // 2      # 4 head pairs
GS = 2            # kt tiles per scores/exp group
F32, F32R, BF16 = mybir.dt.float32, mybir.dt.float32r, mybir.dt.bfloat16
AF = mybir.ActivationFunctionType

_CACHE = {}


def _build(beta_nonzero):
    nc = bacc.Bacc("TRN2", target_bir_lowering=False, debug=False)
    dx = nc.dram_tensor("x", [T, C], F32, kind="ExternalInput")
    dwq = nc.dram_tensor("wq", [KC, 128, 512], BF16, kind="ExternalInput")
    dwk = nc.dram_tensor("wk", [KC, 128, 512], BF16, kind="ExternalInput")
    dwv = nc.dram_tensor("wv", [KC, 128, 512], BF16, kind="ExternalInput")
    dwo = nc.dram_tensor("wo", [NP, 128, 1024], F32R, kind="ExternalInput")
    dmask = nc.dram_tensor("masks", [4, 128, 512], BF16, kind="ExternalInput")
    did = nc.dram_tensor("ident", [128, 128], BF16, kind="ExternalInput")
    dbeta = nc.dram_tensor("betab", [1, C], F32, kind="ExternalInput")
    dout = nc.dram_tensor("out", [T, C], F32, kind="ExternalOutput")

    with tile.TileContext(nc) as tc:
        cst = tc.alloc_tile_pool(name="cst", bufs=1)
        ident = cst.tile([128, 128], BF16)
        mask_sb = cst.tile([128, 4, 512], BF16)
        wo_sb = cst.tile([128, NP, 1024], F32R)
        eps = cst.tile([128, 1], F32)
        nc.sync.dma_start(ident[:], did[:])
        nc.vector.memset(eps[:], 1e-5)
        att = tc.alloc_tile_pool(name="att", bufs=1)
        qT = att.tile([128, NP, T], BF16)
        kT = att.tile([128, NP, T], BF16)
        v_sb = att.tile([128, NT, HC, 65], BF16)
        nc.vector.memset(v_sb[:, :, :, 64:65], 1.0)

        # ---------------- Phase A: LN -> transpose -> QKV ----------------
        with tc.tile_pool(name="wqkv", bufs=1) as wp, \
             tc.tile_pool(name="xp", bufs=3) as xp, \
             tc.tile_pool(name="hp", bufs=3) as hp, \
             tc.tile_pool(name="hT", bufs=1) as hTp, \
             tc.tile_pool(name="st", bufs=4) as stp, \
             tc.tile_pool(name="tps", bufs=3, space="PSUM") as tps, \
             tc.tile_pool(name="qkps", bufs=4, space="PSUM") as qkps:
            wq_sb = wp.tile([128, KC, 512], BF16, tag="w")
            wk_sb = wp.tile([128, KC, 512], BF16, tag="w2")
            wv_sb = wp.tile([128, KC, 512], BF16, tag="w3")
            if beta_nonzero:
                beta_sb = wp.tile([128, C], F32, tag="beta")
                bap = dbeta[0:1, :]
                nc.gpsimd.dma_start(
                    out=beta_sb[:],
                    in_=bass.AP(tensor=bap.tensor, offset=bap.offset,
                                ap=[[0, 128], bap.ap[1]]))
            hT = hTp.tile([128, KC, T], BF16)
            for tb in range(NJ):
                for tt in range(4 * tb, 4 * tb + 4):
                    xt = xp.tile([128, C], F32)
                    nc.sync.dma_start(xt[:], dx[tt * 128:(tt + 1) * 128, :])
                    if tb == 0 and tt < 2:
                        for kc in range(4 * tt, 4 * tt + 4):
                            nc.sync.dma_start(wq_sb[:, kc, :], dwq[kc])
                            nc.sync.dma_start(wk_sb[:, kc, :], dwk[kc])
                            nc.sync.dma_start(wv_sb[:, kc, :], dwv[kc])
                    stats = stp.tile([128, 2, 6], F32, tag="stats")
                    xg = xt[:].rearrange("p (g d) -> p g d", g=2)
                    for g in range(2):
                        nc.vector.bn_stats(stats[:, g, :], xg[:, g, :])
                    mv = stp.tile([128, 2], F32, tag="mv")
                    nc.vector.bn_aggr(mv[:], stats[:])
                    sd = stp.tile([128, 1], F32, tag="sd")
                    nc.scalar.activation(sd[:], mv[:, 1:2], AF.Sqrt, bias=eps[:], scale=1.0)
                    nc.vector.reciprocal(sd[:], sd[:])
                    ht = hp.tile([128, C], BF16)
                    nc.vector.tensor_scalar(
                        out=ht[:], in0=xt[:], scalar1=mv[:, 0:1], scalar2=sd[:],
                        op0=mybir.AluOpType.subtract, op1=mybir.AluOpType.mult)
                    if beta_nonzero:
                        nc.vector.tensor_add(ht[:], ht[:], beta_sb[:])
                    tp = tps.tile([128, KC, 128], BF16)
                    for kc in range(KC):
                        nc.tensor.transpose(tp[:, kc, :], ht[:, kc * 128:(kc + 1) * 128], ident[:])
                    nc.vector.tensor_copy(hT[:, :, tt * 128:(tt + 1) * 128], tp[:])
                for ot in range(NP):
                    pq = qkps.tile([128, 512], F32, tag="ps")
                    for kc in range(KC):
                        nc.tensor.matmul(pq[:], wq_sb[:, kc, ot * 128:(ot + 1) * 128],
                                         hT[:, kc, tb * 512:(tb + 1) * 512],
                                         start=(kc == 0), stop=(kc == KC - 1))
                    nc.vector.tensor_copy(qT[:, ot, tb * 512:(tb + 1) * 512], pq[:])
                    pk = qkps.tile([128, 512], F32, tag="ps")
                    for kc in range(KC):
                        nc.tensor.matmul(pk[:], wk_sb[:, kc, ot * 128:(ot + 1) * 128],
                                         hT[:, kc, tb * 512:(tb + 1) * 512],
                                         start=(kc == 0), stop=(kc == KC - 1))
                    nc.vector.tensor_copy(kT[:, ot, tb * 512:(tb + 1) * 512], pk[:])
                for tt in range(4 * tb, 4 * tb + 4):
                    pv = qkps.tile([128, 512], F32, tag="ps")
                    for kc in range(KC):
                        nc.tensor.matmul(pv[:], hT[:, kc, tt * 128:(tt + 1) * 128],
                                         wv_sb[:, kc, :],
                                         start=(kc == 0), stop=(kc == KC - 1))
                    nc.vector.tensor_copy(
                        v_sb[:, tt, :, 0:64],
                        pv[:].rearrange("p (h d) -> p h d", h=HC))

        # ---------------- Phase B: attention + out_proj ----------------
        with tc.tile_pool(name="sps", bufs=3, space="PSUM") as sps, \
             tc.tile_pool(name="ops", bufs=2, space="PSUM") as ops, \
             tc.tile_pool(name="pp", bufs=6) as ppool, \
             tc.tile_pool(name="up", bufs=10) as upool, \
             tc.tile_pool(name="facp", bufs=8) as facp, \
             tc.tile_pool(name="atmp", bufs=4) as atmp, \
             tc.tile_pool(name="sums", bufs=1) as sums, \
             tc.tile_pool(name="atp", bufs=1) as atp, \
             tc.tile_pool(name="outp", bufs=4) as outp, \
             tc.tile_pool(name="drp", bufs=1, space="DRAM") as drp:
            for r in range(4):
                nc.sync.dma_start(mask_sb[:, r, :], dmask[r])
            for jp in range(NP):
                nc.sync.dma_start(wo_sb[:, jp, :], dwo[jp])
            s8 = sums.tile([8, NJ, 512], F32)
            recip8 = sums.tile([8, NJ, 512], F32)
            AT = atp.tile([128, NP, NJ, 512], F32R)
            drec = drp.tile([8, NJ, 512], F32)

            def emit_attention(J):
                nkt = 4 * J + 4
                u_tiles = []
                for hpair in range(NP):
                    hA, hB = 2 * hpair, 2 * hpair + 1
                    poA = ops.tile([65, 512], F32, tag="po")
                    poB = ops.tile([65, 512], F32, tag="po")
                    for g in range(nkt // GS):
                        kts = list(range(g * GS, (g + 1) * GS))
                        spA = sps.tile([128, GS, 512], F32, tag="sp")
                        spB = sps.tile([128, GS, 512], F32, tag="sp")
                        ptA = ppool.tile([128, GS, 512], BF16, tag="pt")
                        ptB = ppool.tile([128, GS, 512], BF16, tag="pt")
                        # column offset r: tq_loc < r is fully masked for
                        # diagonal tile kt (r = 128*(kt-4J)); skip those columns
                        offs = [max(0, (kt - 4 * J) * 128) for kt in kts]
                        for i, kt in enumerate(kts):
                            for sp, base in ((spA, 0), (spB, 64)):
                                nc.tensor.matmul(
                                    sp[:, i, :],
                                    kT[base:base + 64, hpair, kt * 128:(kt + 1) * 128],
                                    qT[base:base + 64, hpair, J * 512:(J + 1) * 512],
                                    start=True, stop=True,
                                    tile_position=(base, 0))
                        if offs == [0] * GS:
                            for sp, pt in ((spA, ptA), (spB, ptB)):
                                nc.scalar.activation(
                                    pt[:].rearrange("p g f -> p (g f)"),
                                    sp[:].rearrange("p g f -> p (g f)"),
                                    AF.Exp, scale=0.125)
                        else:
                            for i, kt in enumerate(kts):
                                for sp, pt in ((spA, ptA), (spB, ptB)):
                                    nc.scalar.activation(
                                        pt[:, i, offs[i]:512],
                                        sp[:, i, offs[i]:512],
                                        AF.Exp, scale=0.125)
                        for i, kt in enumerate(kts):
                            if kt - 4 * J >= 0:
                                r = offs[i]
                                for pt in (ptA, ptB):
                                    nc.vector.tensor_mul(pt[:, i, r:r + 128],
                                                         pt[:, i, r:r + 128],
                                                         mask_sb[:, 0, 0:128])
                        for i, kt in enumerate(kts):
                            r = offs[i]
                            for po, h, pt in ((poA, hA, ptA), (poB, hB, ptB)):
                                nc.tensor.matmul(
                                    po[:, r:512], v_sb[:, kt, h, :], pt[:, i, r:512],
                                    start=(kt == 0), stop=(kt == nkt - 1))
                    uA = upool.tile([65, 512], F32, tag="u")
                    uB = upool.tile([65, 512], F32, tag="u")
                    nc.vector.tensor_copy(uA[:], poA[:])
                    nc.vector.tensor_copy(uB[:], poB[:])
                    u_tiles.append((uA, uB))
                    nc.sync.dma_start(s8[hA:hA + 1, J, :], uA[64:65, :])
                    nc.sync.dma_start(s8[hB:hB + 1, J, :], uB[64:65, :])
                nc.vector.reciprocal(recip8[:, J, :], s8[:, J, :])
                nc.sync.dma_start(drec[:, J, :], recip8[:, J, :])
                for hpair in range(NP):
                    uA, uB = u_tiles[hpair]
                    for hh, h, u in ((0, 2 * hpair, uA), (1, 2 * hpair + 1, uB)):
                        fac = facp.tile([64, 512], F32)
                        row = drec[h:h + 1, J, :]
                        nc.sync.dma_start(
                            fac[:],
                            bass.AP(tensor=row.tensor, offset=row.offset,
                                    ap=[[0, 64], row.ap[-1]]))
                        if hh == 0:
                            nc.vector.tensor_mul(AT[0:64, hpair, J, :],
                                                 u[0:64, :], fac[:])
                        else:
                            at = atmp.tile([64, 512], F32R)
                            nc.vector.tensor_mul(at[:], u[0:64, :], fac[:])
                            nc.sync.dma_start(AT[64:128, hpair, J, :], at[:])

            def emit_out_proj(J):
                for tc4 in range(4):
                    for ob in range(2):
                        pp_ = sps.tile([128, 512], F32, tag="sp")
                        for hpair in range(NP):
                            nc.tensor.matmul(
                                pp_[:], AT[:, hpair, J, tc4 * 128:(tc4 + 1) * 128],
                                wo_sb[:, hpair, ob * 512:(ob + 1) * 512],
                                start=(hpair == 0), stop=(hpair == NP - 1))
                        ot_ = outp.tile([128, 512], F32)
                        nc.vector.tensor_copy(ot_[:], pp_[:])
                        t0 = J * 512 + tc4 * 128
                        nc.sync.dma_start(dout[t0:t0 + 128, ob * 512:(ob + 1) * 512],
                                          ot_[:])

            for J in range(NJ):
                emit_attention(J)
                if J > 0:
                    emit_out_proj(J - 1)
            emit_out_proj(NJ - 1)
        att.release()
        cst.release()
    nc.compile()
    return nc


def kernel(x, gamma, beta, w_qkv, w_out):
    x = np.asarray(x, dtype=np.float32)
    gamma = np.asarray(gamma, dtype=np.float32)
    beta = np.asarray(beta, dtype=np.float32)
    w_qkv = np.asarray(w_qkv, dtype=np.float32)
    w_out = np.asarray(w_out, dtype=np.float32)
    B = x.shape[0]
    beta_nonzero = bool(np.any(beta != 0.0))
    key = ("k", beta_nonzero)
    if key not in _CACHE:
        _CACHE[key] = _build(beta_nonzero)
    nc = _CACHE[key]

    i128, j128 = np.indices((128, 512))
    masks = np.stack([np.where(i128 + r > j128, 0.0, 1.0)
                      for r in (0, 128, 256, 384)]).astype(ml_dtypes.bfloat16)
    ident = np.eye(128, dtype=ml_dtypes.bfloat16)
    betab = beta.reshape(1, C)

    in_maps = []
    for core in range(8):
        b, g = core // 2, core % 2
        sl = slice(g * 512, (g + 1) * 512)
        wq = (w_qkv[0 * C:1 * C][sl] * gamma[None, :]).T.copy()      # [1024, 512]
        wk = (w_qkv[1 * C:2 * C][sl] * gamma[None, :]).T.copy()
        wv = (w_qkv[2 * C:3 * C][sl] * gamma[None, :]).T.copy()
        wo = w_out[:, sl].T.copy()                                    # [512, 1024]
        in_maps.append({
            "x": np.ascontiguousarray(x[b]),
            "wq": wq.reshape(KC, 128, 512).astype(ml_dtypes.bfloat16),
            "wk": wk.reshape(KC, 128, 512).astype(ml_dtypes.bfloat16),
            "wv": wv.reshape(KC, 128, 512).astype(ml_dtypes.bfloat16),
            "wo": np.ascontiguousarray(wo.reshape(NP, 128, 1024)),
            "masks": masks,
            "ident": ident,
            "betab": betab,
        })
    res = run_bass_kernel_spmd(nc, in_maps, core_ids=list(range(8)))
    out = np.empty((B, T, C), dtype=np.float32)
    for b in range(B):
        out[b] = res.results[2 * b]["out"] + res.results[2 * b + 1]["out"]
    return out



# revision 31
# speedup vs baseline: 1.1615x; 1.1615x over previous
"""CausalSelfAttention TRN2 kernel: LN + QKV + causal attention + out_proj.

Sharding: 8 cores = 4 batches x 2 head-groups (8 heads each). Each core
computes its batch's LayerNorm, QKV for its heads, causal softmax attention,
and a partial out-projection over its heads' channels; the host sums the two
partials per batch.

Design (cost-model driven):
  - hT built via DMA-XBAR transpose (frees PE/DVE), layout [c, tt, kc, t].
  - scores [tk, tq] per 128x512 tile, head-paired via partition ranges
    (tile_position), diagonal tiles column-sliced to skip masked cols.
  - exp on ACT (scale=1/8), fused over GS=2 kt tiles for full tiles.
  - causality: multiplicative [i>j] mask on Pool over both diag tiles of a
    group in one fused stride-trick instruction.
  - PV FLIPPED: out[tq, d] accumulated over kt in PSUM; ones column of v
    gives softmax row-sums per tq partition; per-mtile chains are emitted
    sequentially (PSUM has_written bits are bank-wide on start=True).
  - normalization: per-partition reciprocal + tensor_scalar_mul -> A bf16.
  - A transposed back to [j, t] via DMA-XBAR for the out-projection.
  - Phase A (QKV) and Phase B (attention J blocks) are software-pipelined so
    PE stays busy while ACT exps.
"""
import math
import sys
from collections import deque

sys.path.insert(0, "/opt/trn_rl_repo")
sys.path.insert(0, "/opt/trn_rl_repo/concourse")

import numpy as np
import ml_dtypes

import concourse.bass as bass
import concourse.bacc as bacc
import concourse.mybir as mybir
import concourse.tile as tile
from concourse.bass_utils import run_bass_kernel_spmd

T, C, NH, DH = 2048, 1024, 16, 64
HC = 8            # heads per core
NT = T // 128     # 16 t-tiles
KC = C // 128     # 8 contraction tiles
W = 512           # tq block width
NJ = T // W       # 4 q blocks
NP = HC // 2      # 4 head pairs
GS = 2            # kt tiles per scores/exp group
F32, BF16 = mybir.dt.float32, mybir.dt.bfloat16
AF = mybir.ActivationFunctionType
ALU = mybir.AluOpType

_CACHE = {}


def _build(beta_nonzero):
    nc = bacc.Bacc("TRN2", target_bir_lowering=False, debug=False)
    dx = nc.dram_tensor("x", [T, C], F32, kind="ExternalInput")
    dwq = nc.dram_tensor("wq", [128, KC, 512], BF16, kind="ExternalInput")
    dwk = nc.dram_tensor("wk", [128, KC, 512], BF16, kind="ExternalInput")
    dwv = nc.dram_tensor("wv", [128, KC, 512], BF16, kind="ExternalInput")
    dwo = nc.dram_tensor("wo", [128, NP, 1024], BF16, kind="ExternalInput")
    dmask = nc.dram_tensor("masks", [128, 128], BF16, kind="ExternalInput")
    did = nc.dram_tensor("ident", [128, 128], BF16, kind="ExternalInput")
    dbeta = nc.dram_tensor("betab", [1, C], F32, kind="ExternalInput")
    dout = nc.dram_tensor("out", [T, C], F32, kind="ExternalOutput")

    with tile.TileContext(nc) as tc:
        cst = tc.alloc_tile_pool(name="cst", bufs=1)
        mask_sb = cst.tile([128, 128], BF16)
        wo_sb = cst.tile([128, NP, 1024], BF16)
        wq_sb = cst.tile([128, KC, 512], BF16)
        wk_sb = cst.tile([128, KC, 512], BF16)
        wv_sb = cst.tile([128, KC, 512], BF16)
        eps = cst.tile([128, 1], F32)
        ident = cst.tile([128, 128], BF16)

        att = tc.alloc_tile_pool(name="att", bufs=1)
        hT = att.tile([128, NT, KC, 128], BF16)
        qT = att.tile([128, NP, T], BF16)
        kT = att.tile([128, NP, T], BF16)
        v_sb = att.tile([128, NT, HC, 65], BF16)
        AT = att.tile([128, NJ, 4, NP, 128], BF16)

        nc.vector.memset(eps[:], 1e-5)
        nc.vector.memset(v_sb[:, :, :, 64:65], 1.0)

        with tc.tile_pool(name="xp", bufs=4) as xp, \
             tc.tile_pool(name="stp", bufs=4) as stp, \
             tc.tile_pool(name="hp", bufs=6) as hp, \
             tc.tile_pool(name="ptp", bufs=2) as ptp, \
             tc.tile_pool(name="anp", bufs=2) as anp, \
             tc.tile_pool(name="rcp", bufs=4) as rcp, \
             tc.tile_pool(name="outp", bufs=3) as outp, \
             tc.tile_pool(name="sps", bufs=2, space="PSUM") as sps, \
             tc.tile_pool(name="pvps", bufs=2, space="PSUM") as pvps, \
             tc.tile_pool(name="mmps", bufs=2, space="PSUM") as mmps:

            beta_sb = None
            if beta_nonzero:
                beta_sb = cst.tile([128, C], F32)
                bap = dbeta[0:1, :]
                nc.gpsimd.dma_start(
                    out=beta_sb[:],
                    in_=bass.AP(tensor=bap.tensor, offset=bap.offset,
                                ap=[[0, 128], bap.ap[1]]))

            hts = {}

            def emit_ln_front(tt):
                xt = xp.tile([128, C], F32, tag="x")
                nc.sync.dma_start(xt[:], dx[tt * 128:(tt + 1) * 128, :])
                stats = stp.tile([128, 2, 6], F32, tag="stats")
                xg = xt[:].rearrange("p (g d) -> p g d", g=2)
                for g in range(2):
                    nc.vector.bn_stats(stats[:, g, :], xg[:, g, :])
                mv = stp.tile([128, 2], F32, tag="mv")
                nc.vector.bn_aggr(mv[:], stats[:])
                sd = stp.tile([128, 1], F32, tag="sd")
                nc.scalar.activation(sd[:], mv[:, 1:2], AF.Sqrt, bias=eps[:], scale=1.0)
                nc.vector.reciprocal(sd[:], sd[:])
                ht = hp.tile([128, C], BF16, tag="h")
                nc.gpsimd.tensor_scalar(
                    out=ht[:], in0=xt[:], scalar1=mv[:, 0:1], scalar2=sd[:],
                    op0=ALU.subtract, op1=ALU.mult)
                if beta_nonzero:
                    nc.gpsimd.tensor_add(ht[:], ht[:], beta_sb[:])
                hts[tt] = ht

            def emit_ln_back(tt):
                # PE transpose via identity (DMA-XBAR transposes serialize on
                # the DMA queues/sem channels and wreck the pipeline).
                ht = hts.pop(tt)
                tp = mmps.tile([128, 1024], BF16, tag="mm")
                tpv = tp[:].rearrange("p (k t) -> p k t", k=KC)
                for kc in range(KC):
                    nc.tensor.transpose(tpv[:, kc, :],
                                        ht[:, kc * 128:(kc + 1) * 128], ident[:])
                nc.vector.tensor_copy(hT[:, tt], tpv[:])

            def emit_qk(tb, ot, which):
                w_sb, dstT = (wq_sb, qT) if which == 0 else (wk_sb, kT)
                ps = mmps.tile([128, 512], F32, tag="mm")
                for kc in range(KC):
                    nc.tensor.matmul(ps[:], w_sb[:, kc, ot * 128:(ot + 1) * 128],
                                     hT[:, 4 * tb:4 * tb + 4, kc, :],
                                     start=(kc == 0), stop=(kc == KC - 1))
                nc.vector.tensor_copy(dstT[:, ot, tb * 512:(tb + 1) * 512], ps[:])

            def emit_v(tt):
                ps = mmps.tile([128, 512], F32, tag="mm")
                for kc in range(KC):
                    nc.tensor.matmul(ps[:], hT[:, tt, kc, :], wv_sb[:, kc, :],
                                     start=(kc == 0), stop=(kc == KC - 1))
                nc.vector.tensor_copy(
                    v_sb[:, tt, :, 0:64],
                    ps[:].rearrange("p (h d) -> p h d", h=HC))

            def emit_scores_block(J, h, pt):
                """scores + exp + mask for all kt groups of one head."""
                hp_ = h // 2
                base = 64 * (h % 2)
                nkt = 4 * J + 4
                for g in range(nkt // GS):
                    kts = [GS * g, GS * g + 1]
                    sp = sps.tile([128, GS, 512], F32, tag="sp")
                    for i, kt in enumerate(kts):
                        r = max(0, (kt - 4 * J)) * 128
                        nc.tensor.matmul(
                            sp[:, i, r:512],
                            kT[base:base + 64, hp_, kt * 128:(kt + 1) * 128],
                            qT[base:base + 64, hp_, J * 512 + r:(J + 1) * 512],
                            start=True, stop=True,
                            tile_position=(base, 0))
                    if kts[0] < 4 * J:
                        # both tiles full: single fused exp
                        nc.scalar.activation(
                            pt[:, GS * g:GS * g + GS, :].rearrange("p g f -> p (g f)"),
                            sp[:].rearrange("p g f -> p (g f)"),
                            AF.Exp, scale=0.125)
                    else:
                        # both tiles diagonal: sliced exps + fused 2-slot mask
                        for i, kt in enumerate(kts):
                            r = (kt - 4 * J) * 128
                            nc.scalar.activation(
                                pt[:, GS * g + i, r:512],
                                sp[:, i, r:512],
                                AF.Exp, scale=0.125)
                        r0 = (kts[0] - 4 * J) * 128
                        blk = pt[:, kts[0], r0:r0 + 128]
                        two = bass.AP(tensor=blk.tensor, offset=blk.offset,
                                      ap=[blk.ap[0], [640, 2], [1, 128]])
                        mb = mask_sb[:]
                        mm = bass.AP(tensor=mb.tensor, offset=mb.offset,
                                     ap=[mb.ap[0], [0, 2], [1, 128]])
                        nc.gpsimd.tensor_mul(two, two, mm)

            def emit_pv_block(J, h, pt):
                pv = pvps.tile([128, 4, 128], F32, tag="pv")
                for m in range(4):
                    last = 4 * J + m
                    for kt in range(last + 1):
                        nc.tensor.matmul(
                            pv[:, m, 0:65], pt[:, kt, m * 128:(m + 1) * 128],
                            v_sb[:, kt, h, :],
                            start=(kt == 0), stop=(kt == last))
                return pv

            def emit_norm(J, h, pv, aall):
                rec = rcp.tile([128, 4], F32, tag="rec")
                nc.vector.reciprocal(rec[:], pv[:, :, 64])
                for m in range(4):
                    nc.vector.tensor_scalar_mul(
                        aall[:, m, h * 64:h * 64 + 64],
                        pv[:, m, 0:64],
                        rec[:, m:m + 1])

            def emit_att_transpose(J, aall):
                for m in range(4):
                    tp = mmps.tile([128, 1024], BF16, tag="mm")
                    tpv = tp[:, 0:512].rearrange("p (q t) -> p q t", q=NP)
                    for pr in range(NP):
                        nc.tensor.transpose(
                            tpv[:, pr, :], aall[:, m, pr * 128:(pr + 1) * 128],
                            ident[:])
                    nc.vector.tensor_copy(AT[:, J, m], tpv[:])

            def emit_outproj_chain(J, m, ob):
                ps = mmps.tile([128, 512], F32, tag="mm")
                for p in range(NP):
                    nc.tensor.matmul(
                        ps[:], AT[:, J, m, p, :],
                        wo_sb[:, p, ob * 512:(ob + 1) * 512],
                        start=(p == 0), stop=(p == NP - 1))
                ot_ = outp.tile([128, 512], F32, tag="o")
                nc.vector.tensor_copy(ot_[:], ps[:])
                t0 = J * 512 + m * 128
                nc.sync.dma_start(dout[t0:t0 + 128, ob * 512:(ob + 1) * 512], ot_[:])

            # ---------------- schedule ----------------
            # Two software pipelines:
            #  - PV for head h is emitted after scores for head h+1, so the
            #    PE never waits on exp/mask of the head it just scored.
            #  - rec/norm for a head are deferred one more head so the DVE
            #    reaches them after the PV psum is complete (avoids parking
            #    in the 4-deep wait queue and blocking the DVE sequencer).
            prevs = []     # [(J, h, pt)]   scored, PV not yet emitted
            pending = []   # [(J, h, pv, aall)]  PV emitted, norm not yet

            def flush_pending():
                while pending:
                    emit_norm(*pending.pop(0))

            def pop_pv():
                if prevs:
                    pJ, ph, ppt = prevs.pop(0)
                    pv = emit_pv_block(pJ, ph, ppt)
                    flush_pending()
                    pending.append((pJ, ph, pv, aalls[pJ]))

            def emit_head(J, h, aall, fill):
                pt = ptp.tile([128, NT, 512], BF16, tag="pt")
                emit_scores_block(J, h, pt)
                if fill:
                    fill.popleft()()
                pop_pv()
                prevs.append((J, h, pt))
                if fill:
                    fill.popleft()()

            def flush_heads():
                while prevs:
                    pop_pv()
                flush_pending()

            def qkv_units(tb):
                u = []
                for ot in range(NP):
                    u.append(lambda tb=tb, ot=ot: emit_qk(tb, ot, 0))
                    u.append(lambda tb=tb, ot=ot: emit_qk(tb, ot, 1))
                return u

            def v_units(tb):
                return [lambda tt=tt: emit_v(tt)
                        for tt in range(4 * tb, 4 * tb + 4)]

            def op_units(J):
                return [lambda J=J, m=m, ob=ob: emit_outproj_chain(J, m, ob)
                        for m in range(4) for ob in range(2)]

            def drain(fill):
                while fill:
                    fill.popleft()()

            # s0: x(0..3) lead the DMA device, weights follow on the same
            # queue (no deps, no head-of-line risk), then the LN pipeline
            # rolls: hTt(tt) and x(tt+4) both unblock on LN-ts(tt).
            # All Sqrts stay ahead of the first Exp so the ACT act-table
            # switches only once.
            nc.sync.dma_start(ident[:], did[:])
            emit_ln_front(0)
            emit_ln_front(1)
            nc.sync.dma_start(wv_sb[:], dwv[:])
            emit_ln_front(2)
            emit_ln_front(3)
            nc.sync.dma_start(wq_sb[:], dwq[:])
            nc.sync.dma_start(wk_sb[:], dwk[:])
            ln_fill = deque()
            for tt in range(4, NT):
                ln_fill.append(lambda tt=tt: (emit_ln_back(tt - 4),
                                              emit_ln_front(tt)))
            for tt in range(NT - 4, NT):
                ln_fill.append(lambda tt=tt: emit_ln_back(tt))
            # first 4 transposes + LN rolls before any PE work is possible
            for _ in range(5):
                ln_fill.popleft()()
            for u in v_units(0) + qkv_units(0):
                u()
                if ln_fill:
                    ln_fill.popleft()()
            drain(ln_fill)
            # mask/wo are not needed until s1/s2; scheduling them past the
            # LN pipeline keeps their transfers out of the DMA sem-channel
            # rotation that gates the latency-critical hT transposes.
            with tc.tile_wait_until(0.012):
                nc.scalar.dma_start(mask_sb[:], dmask[:])
            with tc.tile_wait_until(0.022):
                nc.scalar.dma_start(wo_sb[:], dwo[:])

            aalls = {}

            def new_aall(J):
                a_ = anp.tile([128, 4, 512], BF16, tag="aall")
                aalls[J] = a_

            # s1: attn J0; QKV tb=1
            new_aall(0)
            fill = deque(v_units(1) + qkv_units(1))
            for h in range(HC):
                emit_head(0, h, aalls[0], fill)
            drain(fill)

            # s2: attn J1; QKV tb=2 + qk(3); outproj(0).
            # ATt(J-1) is emitted two heads into the next J, once the
            # pipelined PV/norms of J-1 have all been flushed.
            new_aall(1)
            fill = deque(v_units(2) + qkv_units(2) + qkv_units(3) + op_units(0))
            for h in range(HC):
                emit_head(1, h, aalls[1], fill)
                if h == 1:
                    emit_att_transpose(0, aalls[0])
            drain(fill)

            # s3: v(3); attn J2; attn J3 heads 0-1; outproj(1)
            new_aall(2)
            fill = deque(v_units(3) + op_units(1))
            for h in range(HC):
                emit_head(2, h, aalls[2], fill)
                if h == 1:
                    emit_att_transpose(1, aalls[1])
            drain(fill)
            new_aall(3)
            fill = deque()
            for h in range(0, 2):
                emit_head(3, h, aalls[3], fill)
                if h == 1:
                    emit_att_transpose(2, aalls[2])
            drain(fill)

            # s4: attn J3 heads 2-6; outproj(2); head 7 is pipelined per
            # mtile with its norm, AT transpose and outproj so the tail is
            # short.
            fill = deque(op_units(2))
            for h in range(2, HC - 1):
                emit_head(3, h, aalls[3], fill)
            pt7 = ptp.tile([128, NT, 512], BF16, tag="pt")
            emit_scores_block(3, 7, pt7)
            drain(fill)
            flush_heads()
            pv7 = pvps.tile([128, 4, 128], F32, tag="pv")
            rec7 = rcp.tile([128, 4], F32, tag="rec")
            for m in range(4):
                last = 12 + m
                for kt in range(last + 1):
                    nc.tensor.matmul(
                        pv7[:, m, 0:65], pt7[:, kt, m * 128:(m + 1) * 128],
                        v_sb[:, kt, 7, :],
                        start=(kt == 0), stop=(kt == last))
                nc.vector.reciprocal(rec7[:, m:m + 1], pv7[:, m, 64:65])
                nc.vector.tensor_scalar_mul(
                    aalls[3][:, m, 7 * 64:8 * 64], pv7[:, m, 0:64],
                    rec7[:, m:m + 1])
                tp = mmps.tile([128, 1024], BF16, tag="mm")
                tpv = tp[:, 0:512].rearrange("p (q t) -> p q t", q=NP)
                for pr in range(NP):
                    nc.tensor.transpose(
                        tpv[:, pr, :],
                        aalls[3][:, m, pr * 128:(pr + 1) * 128], ident[:])
                nc.vector.tensor_copy(AT[:, 3, m], tpv[:])
                emit_outproj_chain(3, m, 0)
                emit_outproj_chain(3, m, 1)
        att.release()
        cst.release()
    nc.compile()
    return nc


def kernel(x, gamma, beta, w_qkv, w_out):
    x = np.asarray(x, dtype=np.float32)
    gamma = np.asarray(gamma, dtype=np.float32)
    beta = np.asarray(beta, dtype=np.float32)
    w_qkv = np.asarray(w_qkv, dtype=np.float32)
    w_out = np.asarray(w_out, dtype=np.float32)
    B = x.shape[0]
    beta_nonzero = bool(np.any(beta != 0.0))
    key = ("k", beta_nonzero)
    if key not in _CACHE:
        _CACHE[key] = _build(beta_nonzero)
    nc = _CACHE[key]

    i128, j128 = np.indices((128, 128))
    mask = np.where(i128 > j128, 0.0, 1.0).astype(ml_dtypes.bfloat16)
    ident = np.eye(128, dtype=ml_dtypes.bfloat16)
    betab = beta.reshape(1, C)

    def pack_w(w):
        # [1024, 512] -> [128, KC, 512] partition-major
        return np.ascontiguousarray(
            w.reshape(KC, 128, 512).transpose(1, 0, 2)).astype(ml_dtypes.bfloat16)

    in_maps = []
    for core in range(8):
        b, g = core // 2, core % 2
        sl = slice(g * 512, (g + 1) * 512)
        wq = (w_qkv[0 * C:1 * C][sl] * gamma[None, :]).T.copy()      # [1024, 512]
        wk = (w_qkv[1 * C:2 * C][sl] * gamma[None, :]).T.copy()
        wv = (w_qkv[2 * C:3 * C][sl] * gamma[None, :]).T.copy()
        wo = w_out[:, sl].T.copy()                                    # [512, 1024]
        wo_p = np.ascontiguousarray(
            wo.reshape(NP, 128, 1024).transpose(1, 0, 2)).astype(ml_dtypes.bfloat16)
        in_maps.append({
            "x": np.ascontiguousarray(x[b]),
            "wq": pack_w(wq),
            "wk": pack_w(wk),
            "wv": pack_w(wv),
            "wo": wo_p,
            "masks": mask,
            "ident": ident,
            "betab": betab,
        })
    res = run_bass_kernel_spmd(nc, in_maps, core_ids=list(range(8)))
    out = np.empty((B, T, C), dtype=np.float32)
    for b in range(B):
        out[b] = res.results[2 * b]["out"] + res.results[2 * b + 1]["out"]
    return out


# revision 38
# speedup vs baseline: 1.2373x; 1.0653x over previous
"""CausalSelfAttention TRN2 kernel: LN + QKV + causal attention + out_proj.

Sharding: 8 cores = 4 batches x 2 head-groups (8 heads each). Each core
computes its batch's LayerNorm, QKV for its heads, causal softmax attention,
and a partial out-projection over its heads' channels; the host sums the two
partials per batch.

Design (cost-model driven):
  - hT built via DMA-XBAR transpose (frees PE/DVE), layout [c, tt, kc, t].
  - scores [tk, tq] per 128x512 tile, head-paired via partition ranges
    (tile_position), diagonal tiles column-sliced to skip masked cols.
  - exp on ACT (scale=1/8), fused over GS=2 kt tiles for full tiles.
  - causality: multiplicative [i>j] mask on Pool over both diag tiles of a
    group in one fused stride-trick instruction.
  - PV FLIPPED: out[tq, d] accumulated over kt in PSUM; ones column of v
    gives softmax row-sums per tq partition; per-mtile chains are emitted
    sequentially (PSUM has_written bits are bank-wide on start=True).
  - normalization: per-partition reciprocal + tensor_scalar_mul -> A bf16.
  - A transposed back to [j, t] via DMA-XBAR for the out-projection.
  - Phase A (QKV) and Phase B (attention J blocks) are software-pipelined so
    PE stays busy while ACT exps.
"""
import math
import sys
from collections import deque

sys.path.insert(0, "/opt/trn_rl_repo")
sys.path.insert(0, "/opt/trn_rl_repo/concourse")

import numpy as np
import ml_dtypes

import concourse.bass as bass
import concourse.bacc as bacc
import concourse.mybir as mybir
import concourse.tile as tile
from concourse.bass_utils import run_bass_kernel_spmd

T, C, NH, DH = 2048, 1024, 16, 64
HC = 8            # heads per core
NT = T // 128     # 16 t-tiles
KC = C // 128     # 8 contraction tiles
W = 512           # tq block width
NJ = T // W       # 4 q blocks
NP = HC // 2      # 4 head pairs
GS = 2            # kt tiles per scores/exp group
F32, BF16 = mybir.dt.float32, mybir.dt.bfloat16
AF = mybir.ActivationFunctionType
ALU = mybir.AluOpType

_CACHE = {}


def _build(beta_nonzero):
    nc = bacc.Bacc("TRN2", target_bir_lowering=False, debug=False)
    dx = nc.dram_tensor("x", [T, C], BF16, kind="ExternalInput")
    dwq = nc.dram_tensor("wq", [128, KC, 512], BF16, kind="ExternalInput")
    dwk = nc.dram_tensor("wk", [128, KC, 512], BF16, kind="ExternalInput")
    dwv = nc.dram_tensor("wv", [128, KC, 512], BF16, kind="ExternalInput")
    dwo = nc.dram_tensor("wo", [128, NP, 1024], BF16, kind="ExternalInput")
    dmask = nc.dram_tensor("masks", [128, 128], BF16, kind="ExternalInput")
    did = nc.dram_tensor("ident", [128, 128], BF16, kind="ExternalInput")
    dbeta = nc.dram_tensor("betab", [1, C], F32, kind="ExternalInput")
    dout = nc.dram_tensor("out", [T, C], F32, kind="ExternalOutput")

    with tile.TileContext(nc) as tc:
        cst = tc.alloc_tile_pool(name="cst", bufs=1)
        mask_sb = cst.tile([128, 128], BF16)
        wo_sb = cst.tile([128, NP, 1024], BF16)
        wq_sb = cst.tile([128, KC, 512], BF16)
        wk_sb = cst.tile([128, KC, 512], BF16)
        wv_sb = cst.tile([128, KC, 512], BF16)
        eps = cst.tile([128, 1], F32)
        ident = cst.tile([128, 128], BF16)

        att = tc.alloc_tile_pool(name="att", bufs=1)
        hT = att.tile([128, NT, KC, 128], BF16)
        qT = att.tile([128, NP, T], BF16)
        kT = att.tile([128, NP, T], BF16)
        v_sb = att.tile([128, NT, HC, 65], BF16)
        AT = att.tile([128, NJ, 4, NP, 128], BF16)

        nc.vector.memset(eps[:], 1e-5)
        nc.vector.memset(v_sb[:, :, :, 64:65], 1.0)

        with tc.tile_pool(name="xp", bufs=4) as xp, \
             tc.tile_pool(name="stp", bufs=4) as stp, \
             tc.tile_pool(name="hp", bufs=6) as hp, \
             tc.tile_pool(name="ptp", bufs=2) as ptp, \
             tc.tile_pool(name="anp", bufs=3) as anp, \
             tc.tile_pool(name="rcp", bufs=4) as rcp, \
             tc.tile_pool(name="outp", bufs=3) as outp, \
             tc.tile_pool(name="sps", bufs=2, space="PSUM") as sps, \
             tc.tile_pool(name="pvps", bufs=2, space="PSUM") as pvps, \
             tc.tile_pool(name="mmps", bufs=2, space="PSUM") as mmps:

            beta_sb = None
            if beta_nonzero:
                beta_sb = cst.tile([128, C], F32)
                bap = dbeta[0:1, :]
                nc.gpsimd.dma_start(
                    out=beta_sb[:],
                    in_=bass.AP(tensor=bap.tensor, offset=bap.offset,
                                ap=[[0, 128], bap.ap[1]]))

            hts = {}

            def emit_ln_front(tt):
                xt = xp.tile([128, C], BF16, tag="x")
                nc.sync.dma_start(xt[:], dx[tt * 128:(tt + 1) * 128, :])
                stats = stp.tile([128, 2, 6], F32, tag="stats")
                xg = xt[:].rearrange("p (g d) -> p g d", g=2)
                for g in range(2):
                    nc.vector.bn_stats(stats[:, g, :], xg[:, g, :])
                mv = stp.tile([128, 2], F32, tag="mv")
                nc.vector.bn_aggr(mv[:], stats[:])
                sd = stp.tile([128, 1], F32, tag="sd")
                nc.scalar.activation(sd[:], mv[:, 1:2], AF.Sqrt, bias=eps[:], scale=1.0)
                nc.vector.reciprocal(sd[:], sd[:])
                ht = hp.tile([128, C], BF16, tag="h")
                eng = nc.vector if tt < 4 else nc.gpsimd
                eng.tensor_scalar(
                    out=ht[:], in0=xt[:], scalar1=mv[:, 0:1], scalar2=sd[:],
                    op0=ALU.subtract, op1=ALU.mult)
                if beta_nonzero:
                    eng.tensor_add(ht[:], ht[:], beta_sb[:])
                hts[tt] = ht

            def emit_ln_back(tt):
                # PE transpose via identity (DMA-XBAR transposes serialize on
                # the DMA queues/sem channels and wreck the pipeline).
                ht = hts.pop(tt)
                tp = mmps.tile([128, 1024], BF16, tag="mm")
                tpv = tp[:].rearrange("p (k t) -> p k t", k=KC)
                for kc in range(KC):
                    nc.tensor.transpose(tpv[:, kc, :],
                                        ht[:, kc * 128:(kc + 1) * 128], ident[:])
                nc.vector.tensor_copy(hT[:, tt], tpv[:])

            def emit_qk(tb, ot, which):
                w_sb, dstT = (wq_sb, qT) if which == 0 else (wk_sb, kT)
                ps = mmps.tile([128, 512], F32, tag="mm")
                for kc in range(KC):
                    nc.tensor.matmul(ps[:], w_sb[:, kc, ot * 128:(ot + 1) * 128],
                                     hT[:, 4 * tb:4 * tb + 4, kc, :],
                                     start=(kc == 0), stop=(kc == KC - 1))
                nc.vector.tensor_copy(dstT[:, ot, tb * 512:(tb + 1) * 512], ps[:])

            def emit_v(tt):
                ps = mmps.tile([128, 512], F32, tag="mm")
                for kc in range(KC):
                    nc.tensor.matmul(ps[:], hT[:, tt, kc, :], wv_sb[:, kc, :],
                                     start=(kc == 0), stop=(kc == KC - 1))
                nc.vector.tensor_copy(
                    v_sb[:, tt, :, 0:64],
                    ps[:].rearrange("p (h d) -> p h d", h=HC))

            def emit_scores_block(J, h, pt):
                """scores + exp + mask for all kt groups of one head."""
                hp_ = h // 2
                base = 64 * (h % 2)
                nkt = 4 * J + 4
                for g in range(nkt // GS):
                    kts = [GS * g, GS * g + 1]
                    sp = sps.tile([128, GS, 512], F32, tag="sp")
                    for i, kt in enumerate(kts):
                        r = max(0, (kt - 4 * J)) * 128
                        nc.tensor.matmul(
                            sp[:, i, r:512],
                            kT[base:base + 64, hp_, kt * 128:(kt + 1) * 128],
                            qT[base:base + 64, hp_, J * 512 + r:(J + 1) * 512],
                            start=True, stop=True,
                            tile_position=(base, 0))
                    if kts[0] < 4 * J:
                        # both tiles full: single fused exp
                        nc.scalar.activation(
                            pt[:, GS * g:GS * g + GS, :].rearrange("p g f -> p (g f)"),
                            sp[:].rearrange("p g f -> p (g f)"),
                            AF.Exp, scale=0.125)
                    else:
                        # both tiles diagonal: sliced exps + fused 2-slot mask
                        for i, kt in enumerate(kts):
                            r = (kt - 4 * J) * 128
                            nc.scalar.activation(
                                pt[:, GS * g + i, r:512],
                                sp[:, i, r:512],
                                AF.Exp, scale=0.125)
                        r0 = (kts[0] - 4 * J) * 128
                        blk = pt[:, kts[0], r0:r0 + 128]
                        two = bass.AP(tensor=blk.tensor, offset=blk.offset,
                                      ap=[blk.ap[0], [640, 2], [1, 128]])
                        mb = mask_sb[:]
                        mm = bass.AP(tensor=mb.tensor, offset=mb.offset,
                                     ap=[mb.ap[0], [0, 2], [1, 128]])
                        nc.gpsimd.tensor_mul(two, two, mm)

            def emit_pv_block(J, h, pt):
                pv = pvps.tile([128, 4, 128], F32, tag="pv")
                for m in range(4):
                    last = 4 * J + m
                    for kt in range(last + 1):
                        nc.tensor.matmul(
                            pv[:, m, 0:65], pt[:, kt, m * 128:(m + 1) * 128],
                            v_sb[:, kt, h, :],
                            start=(kt == 0), stop=(kt == last))
                return pv

            def emit_norm(J, h, pv, aall):
                rec = rcp.tile([128, 4], F32, tag="rec")
                nc.vector.reciprocal(rec[:], pv[:, :, 64])
                for m in range(4):
                    nc.vector.tensor_scalar_mul(
                        aall[:, m, h * 64:h * 64 + 64],
                        pv[:, m, 0:64],
                        rec[:, m:m + 1])

            def emit_att_transpose(J, aall):
                for m in range(4):
                    tp = mmps.tile([128, 1024], BF16, tag="mm")
                    tpv = tp[:, 0:512].rearrange("p (q t) -> p q t", q=NP)
                    for pr in range(NP):
                        nc.tensor.transpose(
                            tpv[:, pr, :], aall[:, m, pr * 128:(pr + 1) * 128],
                            ident[:])
                    nc.vector.tensor_copy(AT[:, J, m], tpv[:])

            def emit_outproj_chain(J, m, ob):
                ps = mmps.tile([128, 512], F32, tag="mm")
                for p in range(NP):
                    nc.tensor.matmul(
                        ps[:], AT[:, J, m, p, :],
                        wo_sb[:, p, ob * 512:(ob + 1) * 512],
                        start=(p == 0), stop=(p == NP - 1))
                ot_ = outp.tile([128, 512], F32, tag="o")
                nc.vector.tensor_copy(ot_[:], ps[:])
                t0 = J * 512 + m * 128
                nc.sync.dma_start(dout[t0:t0 + 128, ob * 512:(ob + 1) * 512], ot_[:])

            # ---------------- schedule ----------------
            # Two software pipelines:
            #  - PV for head h is emitted after scores for head h+1, so the
            #    PE never waits on exp/mask of the head it just scored.
            #  - rec/norm for a head are deferred one more head so the DVE
            #    reaches them after the PV psum is complete (avoids parking
            #    in the 4-deep wait queue and blocking the DVE sequencer).
            prevs = []     # [(J, h, pt)]   scored, PV not yet emitted
            pending = []   # [(J, h, pv, aall)]  PV emitted, norm not yet

            def flush_pending():
                while pending:
                    emit_norm(*pending.pop(0))

            def pop_pv():
                if prevs:
                    pJ, ph, ppt = prevs.pop(0)
                    pv = emit_pv_block(pJ, ph, ppt)
                    flush_pending()
                    pending.append((pJ, ph, pv, aalls[pJ]))

            def emit_head(J, h, aall, fill):
                pt = ptp.tile([128, NT, 512], BF16, tag="pt")
                emit_scores_block(J, h, pt)
                if fill:
                    fill.popleft()()
                pop_pv()
                prevs.append((J, h, pt))
                if fill:
                    fill.popleft()()

            def flush_heads():
                while prevs:
                    pop_pv()
                flush_pending()

            def qkv_units(tb):
                u = []
                for ot in range(NP):
                    u.append(lambda tb=tb, ot=ot: emit_qk(tb, ot, 0))
                    u.append(lambda tb=tb, ot=ot: emit_qk(tb, ot, 1))
                return u

            def v_units(tb):
                return [lambda tt=tt: emit_v(tt)
                        for tt in range(4 * tb, 4 * tb + 4)]

            def op_units(J):
                return [lambda J=J, m=m, ob=ob: emit_outproj_chain(J, m, ob)
                        for m in range(4) for ob in range(2)]

            def drain(fill):
                while fill:
                    fill.popleft()()

            # s0: x(0..3) lead the DMA device, weights follow on the same
            # queue (no deps, no head-of-line risk), then the LN pipeline
            # rolls: hTt(tt) and x(tt+4) both unblock on LN-ts(tt).
            # All Sqrts stay ahead of the first Exp so the ACT act-table
            # switches only once.
            nc.sync.dma_start(ident[:], did[:])
            emit_ln_front(0)
            emit_ln_front(1)
            nc.sync.dma_start(wv_sb[:], dwv[:])
            emit_ln_front(2)
            emit_ln_front(3)
            nc.sync.dma_start(wq_sb[:], dwq[:])
            nc.sync.dma_start(wk_sb[:], dwk[:])
            ln_fill = deque()
            for tt in range(4, NT):
                ln_fill.append(lambda tt=tt: (emit_ln_back(tt - 4),
                                              emit_ln_front(tt)))
            for tt in range(NT - 4, NT):
                ln_fill.append(lambda tt=tt: emit_ln_back(tt))
            # first 4 transposes + LN rolls before any PE work is possible
            for _ in range(5):
                ln_fill.popleft()()
            for u in v_units(0) + qkv_units(0):
                u()
                if ln_fill:
                    ln_fill.popleft()()
            drain(ln_fill)
            # mask/wo are not needed until s1/s2; scheduling them past the
            # LN pipeline keeps their transfers out of the DMA sem-channel
            # rotation that gates the latency-critical hT transposes.
            with tc.tile_wait_until(0.012):
                nc.scalar.dma_start(mask_sb[:], dmask[:])
            with tc.tile_wait_until(0.022):
                nc.scalar.dma_start(wo_sb[:], dwo[:])

            aalls = {}

            def new_aall(J):
                a_ = anp.tile([128, 4, 512], BF16, tag="aall")
                aalls[J] = a_

            # s1: attn J0; fill: QKV tb=1 and tb=2 (front-loads q/k so J2
            # heads can start mid-s2).
            new_aall(0)
            fill = deque(v_units(1) + qkv_units(1) + qkv_units(2))
            for h in range(HC):
                emit_head(0, h, aalls[0], fill)
            drain(fill)

            # s2: attn J1 with three J2 heads pulled in; fill: v2, qk(3),
            # outproj(0) once ATt(0) lands.
            new_aall(1)
            new_aall(2)
            fill = deque(v_units(2) + qkv_units(3))
            seq2 = [(1, 0), (1, 1), (2, 0), (1, 2), (2, 1), (1, 3), (2, 2),
                    (1, 4), (1, 5), (1, 6), (1, 7)]
            for J, h in seq2:
                emit_head(J, h, aalls[J], fill)
                if (J, h) == (1, 1):
                    emit_att_transpose(0, aalls[0])
                    fill.extend(op_units(0))
            # s3: remaining J2 interleaved with J3; outproj(1)/(2) as late
            # fill; J2's last head is pipelined per mtile with its norm, AT
            # transpose and outproj so the tail overlaps J3h7's exp.
            new_aall(3)
            fill.extend(v_units(3))
            seq3 = [(2, 3), (3, 0), (2, 4), (3, 1), (2, 5), (3, 2), (2, 6),
                    (3, 3), (3, 4), (3, 5), (3, 6), (3, 7)]
            for J, h in seq3:
                emit_head(J, h, aalls[J], fill)
                if (J, h) == (3, 0):
                    # all J1 norms have flushed by now
                    emit_att_transpose(1, aalls[1])
                    fill.extend(op_units(1))
            pt7 = ptp.tile([128, NT, 512], BF16, tag="pt")
            emit_scores_block(2, 7, pt7)
            drain(fill)
            flush_heads()
            emit_att_transpose(3, aalls[3])
            ops3 = deque(op_units(3))
            pv7 = pvps.tile([128, 4, 128], F32, tag="pv")
            rec7 = rcp.tile([128, 4], F32, tag="rec")
            for m in range(4):
                last = 8 + m
                for kt in range(last + 1):
                    nc.tensor.matmul(
                        pv7[:, m, 0:65], pt7[:, kt, m * 128:(m + 1) * 128],
                        v_sb[:, kt, 7, :],
                        start=(kt == 0), stop=(kt == last))
                nc.vector.reciprocal(rec7[:, m:m + 1], pv7[:, m, 64:65])
                nc.vector.tensor_scalar_mul(
                    aalls[2][:, m, 7 * 64:8 * 64], pv7[:, m, 0:64],
                    rec7[:, m:m + 1])
                tp = mmps.tile([128, 1024], BF16, tag="mm")
                tpv = tp[:, 0:512].rearrange("p (q t) -> p q t", q=NP)
                for pr in range(NP):
                    nc.tensor.transpose(
                        tpv[:, pr, :],
                        aalls[2][:, m, pr * 128:(pr + 1) * 128], ident[:])
                nc.vector.tensor_copy(AT[:, 2, m], tpv[:])
                ops3.popleft()()
                ops3.popleft()()
                emit_outproj_chain(2, m, 0)
                emit_outproj_chain(2, m, 1)
        att.release()
        cst.release()
    nc.compile()
    return nc


def kernel(x, gamma, beta, w_qkv, w_out):
    x = np.asarray(x, dtype=np.float32)
    gamma = np.asarray(gamma, dtype=np.float32)
    beta = np.asarray(beta, dtype=np.float32)
    w_qkv = np.asarray(w_qkv, dtype=np.float32)
    w_out = np.asarray(w_out, dtype=np.float32)
    B = x.shape[0]
    beta_nonzero = bool(np.any(beta != 0.0))
    key = ("k", beta_nonzero)
    if key not in _CACHE:
        _CACHE[key] = _build(beta_nonzero)
    nc = _CACHE[key]

    i128, j128 = np.indices((128, 128))
    mask = np.where(i128 > j128, 0.0, 1.0).astype(ml_dtypes.bfloat16)
    ident = np.eye(128, dtype=ml_dtypes.bfloat16)
    betab = beta.reshape(1, C)

    def pack_w(w):
        # [1024, 512] -> [128, KC, 512] partition-major
        return np.ascontiguousarray(
            w.reshape(KC, 128, 512).transpose(1, 0, 2)).astype(ml_dtypes.bfloat16)

    in_maps = []
    for core in range(8):
        b, g = core // 2, core % 2
        sl = slice(g * 512, (g + 1) * 512)
        wq = (w_qkv[0 * C:1 * C][sl] * gamma[None, :]).T.copy()      # [1024, 512]
        wk = (w_qkv[1 * C:2 * C][sl] * gamma[None, :]).T.copy()
        wv = (w_qkv[2 * C:3 * C][sl] * gamma[None, :]).T.copy()
        wo = w_out[:, sl].T.copy()                                    # [512, 1024]
        wo_p = np.ascontiguousarray(
            wo.reshape(NP, 128, 1024).transpose(1, 0, 2)).astype(ml_dtypes.bfloat16)
        in_maps.append({
            "x": np.ascontiguousarray(x[b]).astype(ml_dtypes.bfloat16),
            "wq": pack_w(wq),
            "wk": pack_w(wk),
            "wv": pack_w(wv),
            "wo": wo_p,
            "masks": mask,
            "ident": ident,
            "betab": betab,
        })
    res = run_bass_kernel_spmd(nc, in_maps, core_ids=list(range(8)))
    out = np.empty((B, T, C), dtype=np.float32)
    for b in range(B):
        out[b] = res.results[2 * b]["out"] + res.results[2 * b + 1]["out"]
    return out


# revision 70
# speedup vs baseline: 1.2794x; 1.0341x over previous
"""CausalSelfAttention TRN2 kernel: LN + QKV + causal attention + out_proj.

Sharding: 8 cores = 4 batches x 2 head-groups (8 heads each). Each core
computes its batch's LayerNorm, QKV for its heads, causal softmax attention,
and a partial out-projection over its heads' channels; the host sums the two
partials per batch.

Design (cost-model driven):
  - hT built via DMA-XBAR transpose (frees PE/DVE), layout [c, tt, kc, t].
  - scores [tk, tq] per 128x512 tile, head-paired via partition ranges
    (tile_position), diagonal tiles column-sliced to skip masked cols.
  - exp on ACT (scale=1/8), fused over GS=2 kt tiles for full tiles.
  - causality: multiplicative [i>j] mask on Pool over both diag tiles of a
    group in one fused stride-trick instruction.
  - PV FLIPPED: out[tq, d] accumulated over kt in PSUM; ones column of v
    gives softmax row-sums per tq partition; per-mtile chains are emitted
    sequentially (PSUM has_written bits are bank-wide on start=True).
  - normalization: per-partition reciprocal + tensor_scalar_mul -> A bf16.
  - A transposed back to [j, t] via DMA-XBAR for the out-projection.
  - Phase A (QKV) and Phase B (attention J blocks) are software-pipelined so
    PE stays busy while ACT exps.
"""
import math
import sys
from collections import deque

sys.path.insert(0, "/opt/trn_rl_repo")
sys.path.insert(0, "/opt/trn_rl_repo/concourse")

import numpy as np
import ml_dtypes

import concourse.bass as bass
import concourse.bacc as bacc
import concourse.mybir as mybir
import concourse.tile as tile
from concourse.bass_utils import run_bass_kernel_spmd

T, C, NH, DH = 2048, 1024, 16, 64
HC = 8            # heads per core
NT = T // 128     # 16 t-tiles
KC = C // 128     # 8 contraction tiles
W = 512           # tq block width
NJ = T // W       # 4 q blocks
NP = HC // 2      # 4 head pairs
GS = 2            # kt tiles per scores/exp group
F32, BF16 = mybir.dt.float32, mybir.dt.bfloat16
AF = mybir.ActivationFunctionType
ALU = mybir.AluOpType

_CACHE = {}


def _build(beta_nonzero):
    nc = bacc.Bacc("TRN2", target_bir_lowering=False, debug=False)
    dx = nc.dram_tensor("x", [T, C], BF16, kind="ExternalInput")
    dwq = nc.dram_tensor("wq", [128, KC, 512], BF16, kind="ExternalInput")
    dwk = nc.dram_tensor("wk", [128, KC, 512], BF16, kind="ExternalInput")
    dwv = nc.dram_tensor("wv", [128, KC, 512], BF16, kind="ExternalInput")
    dwo = nc.dram_tensor("wo", [128, NP, 1024], BF16, kind="ExternalInput")
    dmask = nc.dram_tensor("masks", [128, 128], BF16, kind="ExternalInput")
    did = nc.dram_tensor("ident", [128, 128], BF16, kind="ExternalInput")
    dbeta = nc.dram_tensor("betab", [1, C], F32, kind="ExternalInput")
    dout = nc.dram_tensor("out", [T, C], F32, kind="ExternalOutput")

    with tile.TileContext(nc) as tc:
        cst = tc.alloc_tile_pool(name="cst", bufs=1)
        mask_sb = cst.tile([128, 128], BF16)
        wo_sb = cst.tile([128, NP, 1024], BF16)
        wq_sb = cst.tile([128, KC, 512], BF16)
        wk_sb = cst.tile([128, KC, 512], BF16)
        wv_sb = cst.tile([128, KC, 512], BF16)
        eps = cst.tile([128, 1], F32)
        ident = cst.tile([128, 128], BF16)

        att = tc.alloc_tile_pool(name="att", bufs=1)
        hT = att.tile([128, NT, KC, 128], BF16)
        qT = att.tile([128, NP, T], BF16)
        kT = att.tile([128, NP, T], BF16)
        v_sb = att.tile([128, NT, HC, 65], BF16)
        AT = att.tile([128, NJ, 4, NP, 128], BF16)

        nc.vector.memset(eps[:], 1e-5)
        nc.vector.memset(v_sb[:, :, :, 64:65], 1.0)

        with tc.tile_pool(name="xp", bufs=4) as xp, \
             tc.tile_pool(name="stp", bufs=4) as stp, \
             tc.tile_pool(name="hp", bufs=6) as hp, \
             tc.tile_pool(name="ptp", bufs=2) as ptp, \
             tc.tile_pool(name="anp", bufs=3) as anp, \
             tc.tile_pool(name="rcp", bufs=4) as rcp, \
             tc.tile_pool(name="outp", bufs=3) as outp, \
             tc.tile_pool(name="sps", bufs=2, space="PSUM") as sps, \
             tc.tile_pool(name="pvps", bufs=2, space="PSUM") as pvps, \
             tc.tile_pool(name="mmps", bufs=2, space="PSUM") as mmps:

            beta_sb = None
            if beta_nonzero:
                beta_sb = cst.tile([128, C], F32)
                bap = dbeta[0:1, :]
                nc.gpsimd.dma_start(
                    out=beta_sb[:],
                    in_=bass.AP(tensor=bap.tensor, offset=bap.offset,
                                ap=[[0, 128], bap.ap[1]]))

            hts = {}

            def emit_ln_front(tt):
                xt = xp.tile([128, C], BF16, tag="x")
                nc.sync.dma_start(xt[:], dx[tt * 128:(tt + 1) * 128, :])
                stats = stp.tile([128, 2, 6], F32, tag="stats")
                xg = xt[:].rearrange("p (g d) -> p g d", g=2)
                for g in range(2):
                    nc.vector.bn_stats(stats[:, g, :], xg[:, g, :])
                mv = stp.tile([128, 2], F32, tag="mv")
                nc.vector.bn_aggr(mv[:], stats[:])
                sd = stp.tile([128, 1], F32, tag="sd")
                nc.scalar.activation(sd[:], mv[:, 1:2], AF.Sqrt, bias=eps[:], scale=1.0)
                nc.vector.reciprocal(sd[:], sd[:])
                ht = hp.tile([128, C], BF16, tag="h")
                eng = nc.vector if tt < 4 else nc.gpsimd
                eng.tensor_scalar(
                    out=ht[:], in0=xt[:], scalar1=mv[:, 0:1], scalar2=sd[:],
                    op0=ALU.subtract, op1=ALU.mult)
                if beta_nonzero:
                    eng.tensor_add(ht[:], ht[:], beta_sb[:])
                hts[tt] = ht

            def emit_ln_back(tt):
                # PE transpose via identity (DMA-XBAR transposes serialize on
                # the DMA queues/sem channels and wreck the pipeline).
                ht = hts.pop(tt)
                tp = mmps.tile([128, 1024], BF16, tag="mm")
                tpv = tp[:].rearrange("p (k t) -> p k t", k=KC)
                for kc in range(KC):
                    nc.tensor.transpose(tpv[:, kc, :],
                                        ht[:, kc * 128:(kc + 1) * 128], ident[:])
                nc.vector.tensor_copy(hT[:, tt], tpv[:])

            def emit_qk(tb, ot, which):
                w_sb, dstT = (wq_sb, qT) if which == 0 else (wk_sb, kT)
                ps = mmps.tile([128, 512], F32, tag="mm")
                for kc in range(KC):
                    nc.tensor.matmul(ps[:], w_sb[:, kc, ot * 128:(ot + 1) * 128],
                                     hT[:, 4 * tb:4 * tb + 4, kc, :],
                                     start=(kc == 0), stop=(kc == KC - 1))
                nc.vector.tensor_copy(dstT[:, ot, tb * 512:(tb + 1) * 512], ps[:])

            def emit_v(tt):
                ps = mmps.tile([128, 512], F32, tag="mm")
                for kc in range(KC):
                    nc.tensor.matmul(ps[:], hT[:, tt, kc, :], wv_sb[:, kc, :],
                                     start=(kc == 0), stop=(kc == KC - 1))
                nc.vector.tensor_copy(
                    v_sb[:, tt, :, 0:64],
                    ps[:].rearrange("p (h d) -> p h d", h=HC))

            def emit_scores_block(J, h, pt):
                """scores + exp + mask for all kt groups of one head."""
                hp_ = h // 2
                base = 64 * (h % 2)
                nkt = 4 * J + 4
                for g in range(nkt // GS):
                    kts = [GS * g, GS * g + 1]
                    first_diag = kts[0] == 4 * J
                    sp = sps.tile([128, GS, 512], F32, tag="sp")
                    for i, kt in enumerate(kts):
                        r = max(0, (kt - 4 * J)) * 128
                        if first_diag:
                            # computed full-width so the fused exp below reads
                            # only real (finite) scores; the sub-diagonal part
                            # is exp'd but never read by a PV chain.
                            r = 0
                        nc.tensor.matmul(
                            sp[:, i, r:512],
                            kT[base:base + 64, hp_, kt * 128:(kt + 1) * 128],
                            qT[base:base + 64, hp_, J * 512 + r:(J + 1) * 512],
                            start=True, stop=True,
                            tile_position=(base, 0))
                    if kts[0] < 4 * J or first_diag:
                        # both tiles full (or full-computed): one fused exp
                        nc.scalar.activation(
                            pt[:, GS * g:GS * g + GS, :].rearrange("p g f -> p (g f)"),
                            sp[:].rearrange("p g f -> p (g f)"),
                            AF.Exp, scale=0.125)
                    else:
                        # both tiles diagonal: sliced exps
                        for i, kt in enumerate(kts):
                            r = (kt - 4 * J) * 128
                            nc.scalar.activation(
                                pt[:, GS * g + i, r:512],
                                sp[:, i, r:512],
                                AF.Exp, scale=0.125)
                    if kts[0] >= 4 * J:
                        # diagonal group: fused 2-slot [i>j] mask on the two
                        # 128-wide diagonal blocks
                        r0 = (kts[0] - 4 * J) * 128
                        blk = pt[:, kts[0], r0:r0 + 128]
                        two = bass.AP(tensor=blk.tensor, offset=blk.offset,
                                      ap=[blk.ap[0], [640, 2], [1, 128]])
                        mb = mask_sb[:]
                        mm = bass.AP(tensor=mb.tensor, offset=mb.offset,
                                     ap=[mb.ap[0], [0, 2], [1, 128]])
                        nc.vector.tensor_mul(two, two, mm)

            def emit_pv_block(J, h, pt):
                pv = pvps.tile([128, 4, 128], F32, tag="pv")
                for m in range(4):
                    last = 4 * J + m
                    for kt in range(last + 1):
                        nc.tensor.matmul(
                            pv[:, m, 0:65], pt[:, kt, m * 128:(m + 1) * 128],
                            v_sb[:, kt, h, :],
                            start=(kt == 0), stop=(kt == last))
                return pv

            def emit_norm(J, h, pv, aall):
                rec = rcp.tile([128, 4], F32, tag="rec")
                nc.vector.reciprocal(rec[:], pv[:, :, 64])
                for m in range(4):
                    nc.vector.tensor_scalar_mul(
                        aall[:, m, h * 64:h * 64 + 64],
                        pv[:, m, 0:64],
                        rec[:, m:m + 1])

            def emit_att_transpose(J, aall):
                for m in range(4):
                    tp = mmps.tile([128, 1024], BF16, tag="mm")
                    tpv = tp[:, 0:512].rearrange("p (q t) -> p q t", q=NP)
                    for pr in range(NP):
                        nc.tensor.transpose(
                            tpv[:, pr, :], aall[:, m, pr * 128:(pr + 1) * 128],
                            ident[:])
                    nc.vector.tensor_copy(AT[:, J, m], tpv[:])

            def emit_outproj_chain(J, m, ob, q=None):
                ps = mmps.tile([128, 512], F32, tag="mm")
                for p in range(NP):
                    nc.tensor.matmul(
                        ps[:], AT[:, J, m, p, :],
                        wo_sb[:, p, ob * 512:(ob + 1) * 512],
                        start=(p == 0), stop=(p == NP - 1))
                ot_ = outp.tile([128, 512], F32, tag="o")
                nc.vector.tensor_copy(ot_[:], ps[:])
                t0 = J * 512 + m * 128
                (q or nc.sync).dma_start(
                    dout[t0:t0 + 128, ob * 512:(ob + 1) * 512], ot_[:])

            # ---------------- schedule ----------------
            # Two software pipelines:
            #  - PV for head h is emitted after scores for head h+1, so the
            #    PE never waits on exp/mask of the head it just scored.
            #  - rec/norm for a head are deferred one more head so the DVE
            #    reaches them after the PV psum is complete (avoids parking
            #    in the 4-deep wait queue and blocking the DVE sequencer).
            prevs = []     # [(J, h, pt)]   scored, PV not yet emitted
            pending = []   # [(J, h, pv, aall)]  PV emitted, norm not yet

            def flush_pending():
                while pending:
                    emit_norm(*pending.pop(0))

            def pop_pv():
                if prevs:
                    pJ, ph, ppt = prevs.pop(0)
                    pv = emit_pv_block(pJ, ph, ppt)
                    flush_pending()
                    pending.append((pJ, ph, pv, aalls[pJ]))

            def emit_head(J, h, aall, fill):
                pt = ptp.tile([128, NT, 512], BF16, tag="pt")
                emit_scores_block(J, h, pt)
                if fill:
                    fill.popleft()()
                pop_pv()
                prevs.append((J, h, pt))
                if fill:
                    fill.popleft()()

            def flush_heads():
                while prevs:
                    pop_pv()
                flush_pending()

            def qkv_units(tb):
                u = []
                for ot in range(NP):
                    u.append(lambda tb=tb, ot=ot: emit_qk(tb, ot, 0))
                    u.append(lambda tb=tb, ot=ot: emit_qk(tb, ot, 1))
                return u

            def v_units(tb):
                return [lambda tt=tt: emit_v(tt)
                        for tt in range(4 * tb, 4 * tb + 4)]

            def op_units(J):
                return [lambda J=J, m=m, ob=ob: emit_outproj_chain(J, m, ob)
                        for m in range(4) for ob in range(2)]

            def drain(fill):
                while fill:
                    fill.popleft()()

            # s0: x(0..3) lead the DMA device, weights follow on the same
            # queue (no deps, no head-of-line risk), then the LN pipeline
            # rolls: hTt(tt) and x(tt+4) both unblock on LN-ts(tt).
            # All Sqrts stay ahead of the first Exp so the ACT act-table
            # switches only once.
            nc.sync.dma_start(ident[:], did[:])
            emit_ln_front(0)
            emit_ln_front(1)
            nc.sync.dma_start(wv_sb[:], dwv[:])
            emit_ln_front(2)
            emit_ln_front(3)
            nc.sync.dma_start(wq_sb[:], dwq[:])
            nc.sync.dma_start(wk_sb[:], dwk[:])
            # strict (transpose, unit, prefetch) triplets: each PE unit is
            # ring-gated only on the previous tile's transpose copy.
            s0_units = v_units(0) + qkv_units(0)
            for i, u in enumerate(s0_units):
                if i < NT:
                    emit_ln_back(i)
                u()
                if i + 4 < NT:
                    emit_ln_front(i + 4)
            for i in range(len(s0_units), NT):
                emit_ln_back(i)
            # mask/wo are not needed until s1/s2; scheduling them past the
            # LN pipeline keeps their transfers out of the DMA sem-channel
            # rotation that gates the x loads.
            with tc.tile_wait_until(0.012):
                nc.scalar.dma_start(mask_sb[:], dmask[:])
            with tc.tile_wait_until(0.022):
                nc.scalar.dma_start(wo_sb[:], dwo[:])

            aalls = {}

            def new_aall(J):
                a_ = anp.tile([128, 4, 512], BF16, tag="aall")
                aalls[J] = a_

            # s1: attn J0; fill: QKV tb=1
            new_aall(0)
            fill = deque(v_units(1) + qkv_units(1))
            for h in range(HC):
                emit_head(0, h, aalls[0], fill)
            drain(fill)

            # s2: attn J1; fill: v2 + QKV tb=2 + outproj(0). qk(3) is saved
            # for s3 where the ACT-heavy J3 heads need PE fill.
            new_aall(1)
            fill = deque(v_units(2) + qkv_units(2) + op_units(0))
            for h in range(HC):
                emit_head(1, h, aalls[1], fill)
                if h == 1:
                    emit_att_transpose(0, aalls[0])
            drain(fill)

            # s3/s4: J2 heads (PE-surplus) interleaved with J3 heads
            # (ACT-deficit); fill: v3, qk(3) (before J3h0's scores), op1,
            # op2. J3's last head is pipelined per mtile with its norm, AT
            # transpose and outproj so the tail is short.
            new_aall(2)
            new_aall(3)
            fill = deque(v_units(3) + qkv_units(3) + op_units(1))
            seq = [(2, 0), (2, 1), (2, 2), (3, 0), (2, 3), (3, 1), (2, 4),
                   (3, 2), (2, 5), (3, 3), (2, 6), (3, 4), (2, 7), (3, 5),
                   (3, 6)]
            for J, h in seq:
                emit_head(J, h, aalls[J], fill)
                if (J, h) == (2, 1):
                    emit_att_transpose(1, aalls[1])
                if (J, h) == (3, 6):
                    # all J2 norms have flushed by now
                    emit_att_transpose(2, aalls[2])
                    fill.extend(op_units(2))
            pt7 = ptp.tile([128, NT, 512], BF16, tag="pt")
            emit_scores_block(3, 7, pt7)
            drain(fill)
            flush_heads()
            pv7 = pvps.tile([128, 4, 128], F32, tag="pv")
            rec7 = rcp.tile([128, 4], F32, tag="rec")
            for m in range(4):
                last = 12 + m
                for kt in range(last + 1):
                    nc.tensor.matmul(
                        pv7[:, m, 0:65], pt7[:, kt, m * 128:(m + 1) * 128],
                        v_sb[:, kt, 7, :],
                        start=(kt == 0), stop=(kt == last))
                nc.vector.reciprocal(rec7[:, m:m + 1], pv7[:, m, 64:65])
                nc.vector.tensor_scalar_mul(
                    aalls[3][:, m, 7 * 64:8 * 64], pv7[:, m, 0:64],
                    rec7[:, m:m + 1])
                tp = mmps.tile([128, 1024], BF16, tag="mm")
                tpv = tp[:, 0:512].rearrange("p (q t) -> p q t", q=NP)
                for pr in range(NP):
                    nc.tensor.transpose(
                        tpv[:, pr, :],
                        aalls[3][:, m, pr * 128:(pr + 1) * 128], ident[:])
                nc.vector.tensor_copy(AT[:, 3, m], tpv[:])
                emit_outproj_chain(3, m, 0)
                emit_outproj_chain(3, m, 1)
        att.release()
        cst.release()
    nc.compile()
    return nc


def kernel(x, gamma, beta, w_qkv, w_out):
    x = np.asarray(x, dtype=np.float32)
    gamma = np.asarray(gamma, dtype=np.float32)
    beta = np.asarray(beta, dtype=np.float32)
    w_qkv = np.asarray(w_qkv, dtype=np.float32)
    w_out = np.asarray(w_out, dtype=np.float32)
    B = x.shape[0]
    beta_nonzero = bool(np.any(beta != 0.0))
    key = ("k", beta_nonzero)
    if key not in _CACHE:
        _CACHE[key] = _build(beta_nonzero)
    nc = _CACHE[key]

    i128, j128 = np.indices((128, 128))
    mask = np.where(i128 > j128, 0.0, 1.0).astype(ml_dtypes.bfloat16)
    ident = np.eye(128, dtype=ml_dtypes.bfloat16)
    betab = beta.reshape(1, C)

    def pack_w(w):
        # [1024, 512] -> [128, KC, 512] partition-major
        return np.ascontiguousarray(
            w.reshape(KC, 128, 512).transpose(1, 0, 2)).astype(ml_dtypes.bfloat16)

    in_maps = []
    for core in range(8):
        b, g = core // 2, core % 2
        sl = slice(g * 512, (g + 1) * 512)
        wq = (w_qkv[0 * C:1 * C][sl] * gamma[None, :]).T.copy()      # [1024, 512]
        wk = (w_qkv[1 * C:2 * C][sl] * gamma[None, :]).T.copy()
        wv = (w_qkv[2 * C:3 * C][sl] * gamma[None, :]).T.copy()
        wo = w_out[:, sl].T.copy()                                    # [512, 1024]
        wo_p = np.ascontiguousarray(
            wo.reshape(NP, 128, 1024).transpose(1, 0, 2)).astype(ml_dtypes.bfloat16)
        in_maps.append({
            "x": np.ascontiguousarray(x[b]).astype(ml_dtypes.bfloat16),
            "wq": pack_w(wq),
            "wk": pack_w(wk),
            "wv": pack_w(wv),
            "wo": wo_p,
            "masks": mask,
            "ident": ident,
            "betab": betab,
        })
    res = run_bass_kernel_spmd(nc, in_maps, core_ids=list(range(8)))
    out = np.empty((B, T, C), dtype=np.float32)
    for b in range(B):
        out[b] = res.results[2 * b]["out"] + res.results[2 * b + 1]["out"]
    return out


# revision 71
# speedup vs baseline: 1.2894x; 1.0077x over previous
"""CausalSelfAttention TRN2 kernel: LN + QKV + causal attention + out_proj.

Sharding: 8 cores = 4 batches x 2 head-groups (8 heads each). Each core
computes its batch's LayerNorm, QKV for its heads, causal softmax attention,
and a partial out-projection over its heads' channels; the host sums the two
partials per batch.

Design (cost-model driven):
  - hT built via DMA-XBAR transpose (frees PE/DVE), layout [c, tt, kc, t].
  - scores [tk, tq] per 128x512 tile, head-paired via partition ranges
    (tile_position), diagonal tiles column-sliced to skip masked cols.
  - exp on ACT (scale=1/8), fused over GS=2 kt tiles for full tiles.
  - causality: multiplicative [i>j] mask on Pool over both diag tiles of a
    group in one fused stride-trick instruction.
  - PV FLIPPED: out[tq, d] accumulated over kt in PSUM; ones column of v
    gives softmax row-sums per tq partition; per-mtile chains are emitted
    sequentially (PSUM has_written bits are bank-wide on start=True).
  - normalization: per-partition reciprocal + tensor_scalar_mul -> A bf16.
  - A transposed back to [j, t] via DMA-XBAR for the out-projection.
  - Phase A (QKV) and Phase B (attention J blocks) are software-pipelined so
    PE stays busy while ACT exps.
"""
import math
import sys
from collections import deque

sys.path.insert(0, "/opt/trn_rl_repo")
sys.path.insert(0, "/opt/trn_rl_repo/concourse")

import numpy as np
import ml_dtypes

import concourse.bass as bass
import concourse.bacc as bacc
import concourse.mybir as mybir
import concourse.tile as tile
from concourse.bass_utils import run_bass_kernel_spmd

T, C, NH, DH = 2048, 1024, 16, 64
HC = 8            # heads per core
NT = T // 128     # 16 t-tiles
KC = C // 128     # 8 contraction tiles
W = 512           # tq block width
NJ = T // W       # 4 q blocks
NP = HC // 2      # 4 head pairs
GS = 2            # kt tiles per scores/exp group
F32, BF16 = mybir.dt.float32, mybir.dt.bfloat16
AF = mybir.ActivationFunctionType
ALU = mybir.AluOpType

_CACHE = {}


def _build(beta_nonzero):
    nc = bacc.Bacc("TRN2", target_bir_lowering=False, debug=False)
    dx = nc.dram_tensor("x", [T, C], BF16, kind="ExternalInput")
    dwq = nc.dram_tensor("wq", [128, KC, 512], BF16, kind="ExternalInput")
    dwk = nc.dram_tensor("wk", [128, KC, 512], BF16, kind="ExternalInput")
    dwv = nc.dram_tensor("wv", [128, KC, 512], BF16, kind="ExternalInput")
    dwo = nc.dram_tensor("wo", [128, NP, 1024], BF16, kind="ExternalInput")
    dmask = nc.dram_tensor("masks", [128, 128], BF16, kind="ExternalInput")
    did = nc.dram_tensor("ident", [128, 128], BF16, kind="ExternalInput")
    dbeta = nc.dram_tensor("betab", [1, C], F32, kind="ExternalInput")
    dout = nc.dram_tensor("out", [T, C], F32, kind="ExternalOutput")

    with tile.TileContext(nc) as tc:
        cst = tc.alloc_tile_pool(name="cst", bufs=1)
        mask_sb = cst.tile([128, 128], BF16)
        wo_sb = cst.tile([128, NP, 1024], BF16)
        wq_sb = cst.tile([128, KC, 512], BF16)
        wk_sb = cst.tile([128, KC, 512], BF16)
        wv_sb = cst.tile([128, KC, 512], BF16)
        eps = cst.tile([128, 1], F32)
        ident = cst.tile([128, 128], BF16)

        att = tc.alloc_tile_pool(name="att", bufs=1)
        hT = att.tile([128, NT, KC, 128], BF16)
        qT = att.tile([128, NP, T], BF16)
        kT = att.tile([128, NP, T], BF16)
        v_sb = att.tile([128, NT, HC, 65], BF16)
        AT = att.tile([128, NJ, 4, NP, 128], BF16)

        nc.vector.memset(eps[:], 1e-5)
        nc.vector.memset(v_sb[:, :, :, 64:65], 1.0)

        with tc.tile_pool(name="xp", bufs=5) as xp, \
             tc.tile_pool(name="stp", bufs=4) as stp, \
             tc.tile_pool(name="hp", bufs=6) as hp, \
             tc.tile_pool(name="ptp", bufs=2) as ptp, \
             tc.tile_pool(name="anp", bufs=3) as anp, \
             tc.tile_pool(name="rcp", bufs=4) as rcp, \
             tc.tile_pool(name="outp", bufs=3) as outp, \
             tc.tile_pool(name="sps", bufs=2, space="PSUM") as sps, \
             tc.tile_pool(name="pvps", bufs=2, space="PSUM") as pvps, \
             tc.tile_pool(name="mmps", bufs=2, space="PSUM") as mmps:

            beta_sb = None
            if beta_nonzero:
                beta_sb = cst.tile([128, C], F32)
                bap = dbeta[0:1, :]
                nc.gpsimd.dma_start(
                    out=beta_sb[:],
                    in_=bass.AP(tensor=bap.tensor, offset=bap.offset,
                                ap=[[0, 128], bap.ap[1]]))

            hts = {}

            def emit_ln_front(tt):
                xt = xp.tile([128, C], BF16, tag="x")
                nc.sync.dma_start(xt[:], dx[tt * 128:(tt + 1) * 128, :])
                stats = stp.tile([128, 2, 6], F32, tag="stats")
                xg = xt[:].rearrange("p (g d) -> p g d", g=2)
                for g in range(2):
                    nc.vector.bn_stats(stats[:, g, :], xg[:, g, :])
                mv = stp.tile([128, 2], F32, tag="mv")
                nc.vector.bn_aggr(mv[:], stats[:])
                sd = stp.tile([128, 1], F32, tag="sd")
                nc.scalar.activation(sd[:], mv[:, 1:2], AF.Sqrt, bias=eps[:], scale=1.0)
                nc.vector.reciprocal(sd[:], sd[:])
                ht = hp.tile([128, C], BF16, tag="h")
                eng = nc.vector if tt < 4 else nc.gpsimd
                eng.tensor_scalar(
                    out=ht[:], in0=xt[:], scalar1=mv[:, 0:1], scalar2=sd[:],
                    op0=ALU.subtract, op1=ALU.mult)
                if beta_nonzero:
                    eng.tensor_add(ht[:], ht[:], beta_sb[:])
                hts[tt] = ht

            def emit_ln_back(tt):
                # PE transpose via identity (DMA-XBAR transposes serialize on
                # the DMA queues/sem channels and wreck the pipeline).
                ht = hts.pop(tt)
                tp = mmps.tile([128, 1024], BF16, tag="mm")
                tpv = tp[:].rearrange("p (k t) -> p k t", k=KC)
                for kc in range(KC):
                    nc.tensor.transpose(tpv[:, kc, :],
                                        ht[:, kc * 128:(kc + 1) * 128], ident[:])
                nc.vector.tensor_copy(hT[:, tt], tpv[:])

            def emit_qk(tb, ot, which):
                w_sb, dstT = (wq_sb, qT) if which == 0 else (wk_sb, kT)
                ps = mmps.tile([128, 512], F32, tag="mm")
                for kc in range(KC):
                    nc.tensor.matmul(ps[:], w_sb[:, kc, ot * 128:(ot + 1) * 128],
                                     hT[:, 4 * tb:4 * tb + 4, kc, :],
                                     start=(kc == 0), stop=(kc == KC - 1))
                nc.vector.tensor_copy(dstT[:, ot, tb * 512:(tb + 1) * 512], ps[:])

            def emit_v(tt):
                ps = mmps.tile([128, 512], F32, tag="mm")
                for kc in range(KC):
                    nc.tensor.matmul(ps[:], hT[:, tt, kc, :], wv_sb[:, kc, :],
                                     start=(kc == 0), stop=(kc == KC - 1))
                nc.vector.tensor_copy(
                    v_sb[:, tt, :, 0:64],
                    ps[:].rearrange("p (h d) -> p h d", h=HC))

            def emit_scores_block(J, h, pt):
                """scores + exp + mask for all kt groups of one head."""
                hp_ = h // 2
                base = 64 * (h % 2)
                nkt = 4 * J + 4
                for g in range(nkt // GS):
                    kts = [GS * g, GS * g + 1]
                    first_diag = kts[0] == 4 * J
                    sp = sps.tile([128, GS, 512], F32, tag="sp")
                    for i, kt in enumerate(kts):
                        r = max(0, (kt - 4 * J)) * 128
                        if first_diag:
                            # computed full-width so the fused exp below reads
                            # only real (finite) scores; the sub-diagonal part
                            # is exp'd but never read by a PV chain.
                            r = 0
                        nc.tensor.matmul(
                            sp[:, i, r:512],
                            kT[base:base + 64, hp_, kt * 128:(kt + 1) * 128],
                            qT[base:base + 64, hp_, J * 512 + r:(J + 1) * 512],
                            start=True, stop=True,
                            tile_position=(base, 0))
                    if kts[0] < 4 * J or first_diag:
                        # both tiles full (or full-computed): one fused exp
                        nc.scalar.activation(
                            pt[:, GS * g:GS * g + GS, :].rearrange("p g f -> p (g f)"),
                            sp[:].rearrange("p g f -> p (g f)"),
                            AF.Exp, scale=0.125)
                    else:
                        # both tiles diagonal: sliced exps
                        for i, kt in enumerate(kts):
                            r = (kt - 4 * J) * 128
                            nc.scalar.activation(
                                pt[:, GS * g + i, r:512],
                                sp[:, i, r:512],
                                AF.Exp, scale=0.125)
                    if kts[0] >= 4 * J:
                        # diagonal group: fused 2-slot [i>j] mask on the two
                        # 128-wide diagonal blocks
                        r0 = (kts[0] - 4 * J) * 128
                        blk = pt[:, kts[0], r0:r0 + 128]
                        two = bass.AP(tensor=blk.tensor, offset=blk.offset,
                                      ap=[blk.ap[0], [640, 2], [1, 128]])
                        mb = mask_sb[:]
                        mm = bass.AP(tensor=mb.tensor, offset=mb.offset,
                                     ap=[mb.ap[0], [0, 2], [1, 128]])
                        nc.vector.tensor_mul(two, two, mm)

            def emit_pv_block(J, h, pt):
                pv = pvps.tile([128, 4, 128], F32, tag="pv")
                for m in range(4):
                    last = 4 * J + m
                    for kt in range(last + 1):
                        nc.tensor.matmul(
                            pv[:, m, 0:65], pt[:, kt, m * 128:(m + 1) * 128],
                            v_sb[:, kt, h, :],
                            start=(kt == 0), stop=(kt == last))
                return pv

            def emit_norm(J, h, pv, aall):
                rec = rcp.tile([128, 4], F32, tag="rec")
                nc.vector.reciprocal(rec[:], pv[:, :, 64])
                for m in range(4):
                    nc.vector.tensor_scalar_mul(
                        aall[:, m, h * 64:h * 64 + 64],
                        pv[:, m, 0:64],
                        rec[:, m:m + 1])

            def emit_att_transpose(J, aall):
                for m in range(4):
                    tp = mmps.tile([128, 1024], BF16, tag="mm")
                    tpv = tp[:, 0:512].rearrange("p (q t) -> p q t", q=NP)
                    for pr in range(NP):
                        nc.tensor.transpose(
                            tpv[:, pr, :], aall[:, m, pr * 128:(pr + 1) * 128],
                            ident[:])
                    nc.vector.tensor_copy(AT[:, J, m], tpv[:])

            def emit_outproj_chain(J, m, ob, q=None):
                ps = mmps.tile([128, 512], F32, tag="mm")
                for p in range(NP):
                    nc.tensor.matmul(
                        ps[:], AT[:, J, m, p, :],
                        wo_sb[:, p, ob * 512:(ob + 1) * 512],
                        start=(p == 0), stop=(p == NP - 1))
                ot_ = outp.tile([128, 512], F32, tag="o")
                nc.vector.tensor_copy(ot_[:], ps[:])
                t0 = J * 512 + m * 128
                (q or nc.sync).dma_start(
                    dout[t0:t0 + 128, ob * 512:(ob + 1) * 512], ot_[:])

            # ---------------- schedule ----------------
            # Two software pipelines:
            #  - PV for head h is emitted after scores for head h+1, so the
            #    PE never waits on exp/mask of the head it just scored.
            #  - rec/norm for a head are deferred one more head so the DVE
            #    reaches them after the PV psum is complete (avoids parking
            #    in the 4-deep wait queue and blocking the DVE sequencer).
            prevs = []     # [(J, h, pt)]   scored, PV not yet emitted
            pending = []   # [(J, h, pv, aall)]  PV emitted, norm not yet

            def flush_pending():
                while pending:
                    emit_norm(*pending.pop(0))

            def pop_pv():
                if prevs:
                    pJ, ph, ppt = prevs.pop(0)
                    pv = emit_pv_block(pJ, ph, ppt)
                    flush_pending()
                    pending.append((pJ, ph, pv, aalls[pJ]))

            def emit_head(J, h, aall, fill):
                pt = ptp.tile([128, NT, 512], BF16, tag="pt")
                emit_scores_block(J, h, pt)
                if fill:
                    fill.popleft()()
                pop_pv()
                prevs.append((J, h, pt))
                if fill:
                    fill.popleft()()

            def flush_heads():
                while prevs:
                    pop_pv()
                flush_pending()

            def qkv_units(tb):
                u = []
                for ot in range(NP):
                    u.append(lambda tb=tb, ot=ot: emit_qk(tb, ot, 0))
                    u.append(lambda tb=tb, ot=ot: emit_qk(tb, ot, 1))
                return u

            def v_units(tb):
                return [lambda tt=tt: emit_v(tt)
                        for tt in range(4 * tb, 4 * tb + 4)]

            def op_units(J):
                return [lambda J=J, m=m, ob=ob: emit_outproj_chain(J, m, ob)
                        for m in range(4) for ob in range(2)]

            def drain(fill):
                while fill:
                    fill.popleft()()

            # s0: x(0..3) lead the DMA device, weights follow on the same
            # queue (no deps, no head-of-line risk), then the LN pipeline
            # rolls: hTt(tt) and x(tt+4) both unblock on LN-ts(tt).
            # All Sqrts stay ahead of the first Exp so the ACT act-table
            # switches only once.
            nc.sync.dma_start(ident[:], did[:])
            emit_ln_front(0)
            emit_ln_front(1)
            nc.sync.dma_start(wv_sb[:], dwv[:])
            emit_ln_front(2)
            emit_ln_front(3)
            nc.sync.dma_start(wq_sb[:], dwq[:])
            nc.sync.dma_start(wk_sb[:], dwk[:])
            # strict (transpose, unit, prefetch) triplets: each PE unit is
            # ring-gated only on the previous tile's transpose copy.
            s0_units = v_units(0) + qkv_units(0)
            for i, u in enumerate(s0_units):
                if i < NT:
                    emit_ln_back(i)
                u()
                if i + 4 < NT:
                    emit_ln_front(i + 4)
            for i in range(len(s0_units), NT):
                emit_ln_back(i)
            # mask/wo are not needed until s1/s2; scheduling them past the
            # LN pipeline keeps their transfers out of the DMA sem-channel
            # rotation that gates the x loads.
            with tc.tile_wait_until(0.012):
                nc.scalar.dma_start(mask_sb[:], dmask[:])
            with tc.tile_wait_until(0.022):
                nc.scalar.dma_start(wo_sb[:], dwo[:])

            aalls = {}

            def new_aall(J):
                a_ = anp.tile([128, 4, 512], BF16, tag="aall")
                aalls[J] = a_

            # s1: attn J0; fill: QKV tb=1
            new_aall(0)
            fill = deque(v_units(1) + qkv_units(1))
            for h in range(HC):
                emit_head(0, h, aalls[0], fill)
            drain(fill)

            # s2: attn J1; fill: v2 + QKV tb=2 + outproj(0). qk(3) is saved
            # for s3 where the ACT-heavy J3 heads need PE fill.
            new_aall(1)
            fill = deque(v_units(2) + qkv_units(2) + op_units(0))
            for h in range(HC):
                emit_head(1, h, aalls[1], fill)
                if h == 1:
                    emit_att_transpose(0, aalls[0])
            drain(fill)

            # s3/s4: J2 heads (PE-surplus) interleaved with J3 heads
            # (ACT-deficit); fill: v3, qk(3) (before J3h0's scores), op1,
            # op2. J3's last head is pipelined per mtile with its norm, AT
            # transpose and outproj so the tail is short.
            new_aall(2)
            new_aall(3)
            fill = deque(v_units(3) + qkv_units(3) + op_units(1))
            seq = [(2, 0), (2, 1), (2, 2), (3, 0), (2, 3), (3, 1), (2, 4),
                   (3, 2), (2, 5), (3, 3), (2, 6), (3, 4), (2, 7), (3, 5),
                   (3, 6)]
            for J, h in seq:
                emit_head(J, h, aalls[J], fill)
                if (J, h) == (2, 1):
                    emit_att_transpose(1, aalls[1])
                if (J, h) == (3, 6):
                    # all J2 norms have flushed by now
                    emit_att_transpose(2, aalls[2])
                    fill.extend(op_units(2))
            pt7 = ptp.tile([128, NT, 512], BF16, tag="pt")
            emit_scores_block(3, 7, pt7)
            drain(fill)
            flush_heads()
            pv7 = pvps.tile([128, 4, 128], F32, tag="pv")
            rec7 = rcp.tile([128, 4], F32, tag="rec")
            for m in range(4):
                last = 12 + m
                for kt in range(last + 1):
                    nc.tensor.matmul(
                        pv7[:, m, 0:65], pt7[:, kt, m * 128:(m + 1) * 128],
                        v_sb[:, kt, 7, :],
                        start=(kt == 0), stop=(kt == last))
                nc.vector.reciprocal(rec7[:, m:m + 1], pv7[:, m, 64:65])
                nc.vector.tensor_scalar_mul(
                    aalls[3][:, m, 7 * 64:8 * 64], pv7[:, m, 0:64],
                    rec7[:, m:m + 1])
                tp = mmps.tile([128, 1024], BF16, tag="mm")
                tpv = tp[:, 0:512].rearrange("p (q t) -> p q t", q=NP)
                for pr in range(NP):
                    nc.tensor.transpose(
                        tpv[:, pr, :],
                        aalls[3][:, m, pr * 128:(pr + 1) * 128], ident[:])
                nc.vector.tensor_copy(AT[:, 3, m], tpv[:])
                emit_outproj_chain(3, m, 0)
                emit_outproj_chain(3, m, 1)
        att.release()
        cst.release()
    nc.compile()
    return nc


def kernel(x, gamma, beta, w_qkv, w_out):
    x = np.asarray(x, dtype=np.float32)
    gamma = np.asarray(gamma, dtype=np.float32)
    beta = np.asarray(beta, dtype=np.float32)
    w_qkv = np.asarray(w_qkv, dtype=np.float32)
    w_out = np.asarray(w_out, dtype=np.float32)
    B = x.shape[0]
    beta_nonzero = bool(np.any(beta != 0.0))
    key = ("k", beta_nonzero)
    if key not in _CACHE:
        _CACHE[key] = _build(beta_nonzero)
    nc = _CACHE[key]

    i128, j128 = np.indices((128, 128))
    mask = np.where(i128 > j128, 0.0, 1.0).astype(ml_dtypes.bfloat16)
    ident = np.eye(128, dtype=ml_dtypes.bfloat16)
    betab = beta.reshape(1, C)

    def pack_w(w):
        # [1024, 512] -> [128, KC, 512] partition-major
        return np.ascontiguousarray(
            w.reshape(KC, 128, 512).transpose(1, 0, 2)).astype(ml_dtypes.bfloat16)

    in_maps = []
    for core in range(8):
        b, g = core // 2, core % 2
        sl = slice(g * 512, (g + 1) * 512)
        wq = (w_qkv[0 * C:1 * C][sl] * gamma[None, :]).T.copy()      # [1024, 512]
        wk = (w_qkv[1 * C:2 * C][sl] * gamma[None, :]).T.copy()
        wv = (w_qkv[2 * C:3 * C][sl] * gamma[None, :]).T.copy()
        wo = w_out[:, sl].T.copy()                                    # [512, 1024]
        wo_p = np.ascontiguousarray(
            wo.reshape(NP, 128, 1024).transpose(1, 0, 2)).astype(ml_dtypes.bfloat16)
        in_maps.append({
            "x": np.ascontiguousarray(x[b]).astype(ml_dtypes.bfloat16),
            "wq": pack_w(wq),
            "wk": pack_w(wk),
            "wv": pack_w(wv),
            "wo": wo_p,
            "masks": mask,
            "ident": ident,
            "betab": betab,
        })
    res = run_bass_kernel_spmd(nc, in_maps, core_ids=list(range(8)))
    out = np.empty((B, T, C), dtype=np.float32)
    for b in range(B):
        out[b] = res.results[2 * b]["out"] + res.results[2 * b + 1]["out"]
    return out
